# revision 15
# baseline (speedup 1.0000x reference)
"""NNUE forward kernel for Trainium2 (8 NeuronCores, batch-parallel).

Network (per batch row b, dual perspective p in {0,1}):
    a_p = relu(x[b,p,:] @ W1.T + b1)        # [256], K = 45056
    a   = concat(a_0, a_1)                  # [512]
    h   = relu(a @ W2.T + b2)               # [32]
    h2  = relu(h @ W3.T + b3)               # [32]
    y   = sigmoid(h2 @ W4.T + b4)           # [1]

Sharding: data-parallel over batch.  Each core takes 256 batch rows
(= 512 matmul columns counting both perspectives, which share W1).  W1
and the tiny MLP weights are replicated.  All matmul operands are cast
to bf16 on the host (fp32 PSUM accumulation on device), which halves
HBM traffic and runs the PE at full bf16 rate.

On-device layout: features live on the partition axis (prepared
host-side, partition-major in DRAM so every DMA is one contiguous run
per partition).  The front layer is out[o, col] = sum_f W1T[f, o] *
xT[f, col], 352 K-tiles of 128 accumulating into two PSUM banks
([128 outs, 512 cols] each).  The K stream is chunked with small chunks
first so the PE starts as early as possible.  The MLP tail keeps batch
on the free axis, so no on-device transposes are needed anywhere.
"""

import numpy as np
import ml_dtypes

BF16 = ml_dtypes.bfloat16
FP8 = ml_dtypes.float8_e4m3
W1_SCALE = 256.0  # W1 ~ 1/sqrt(45056) sits near fp8 subnormals; pre-scale up

N_CORES = 8

# Full-problem geometry (hardcoded per the harness contract).
B = 2048          # total batch
PERSP = 2
F = 45056         # input features = 352 * 128
O1 = 256          # front layer width
NKT = F // 128    # 352 K-tiles
BC = B // N_CORES    # 256 batch rows per core
NCOL = BC * PERSP    # 512 matmul columns per core
# K-tiles per DMA chunk: small chunks first to cut PE startup latency,
# then big chunks so each fp8 transfer stays >= 1-2 MB for DMA efficiency.
CHUNKS = [4, 4, 8, 16, 16, 16] + [32] * ((NKT - 64) // 32)
assert sum(CHUNKS) == NKT


def _build(chunks, nkt, ncol, num_devices=N_CORES):
    """Emit the Bass program. Returns the compiled Bacc object."""
    import concourse.mybir as mybir
    from concourse import bacc
    from concourse.tile import TileContext

    f32 = mybir.dt.float32
    bf16 = mybir.dt.bfloat16
    fp8 = mybir.dt.float8e4
    Act = mybir.ActivationFunctionType

    half = ncol // 2   # columns per perspective
    cmax = max(chunks)

    nc = bacc.Bacc(
        "TRN2", target_bir_lowering=False, debug=False, num_devices=num_devices
    )

    # Chunk blocks are packed sequentially in DRAM (each block is
    # [128, cnt, ncol] flattened row-major) so every streaming DMA reads
    # one dense region of HBM.
    x_d = nc.dram_tensor("x", [nkt * 128, ncol], fp8, kind="ExternalInput")
    w_d = nc.dram_tensor("w1", [nkt * 128, O1], fp8, kind="ExternalInput")
    b1_d = nc.dram_tensor("b1", [128, 2], f32, kind="ExternalInput")
    w2_d = nc.dram_tensor("w2", [128, 128], bf16, kind="ExternalInput")
    b2_d = nc.dram_tensor("b2", [32, 1], f32, kind="ExternalInput")
    w3_d = nc.dram_tensor("w3", [32, 32], bf16, kind="ExternalInput")
    b3_d = nc.dram_tensor("b3", [32, 1], f32, kind="ExternalInput")
    w4_d = nc.dram_tensor("w4", [32, 1], bf16, kind="ExternalInput")
    b4_d = nc.dram_tensor("b4", [1, 1], f32, kind="ExternalInput")
    out_d = nc.dram_tensor("out", [1, half], f32, kind="ExternalOutput")

    with TileContext(nc) as tc, tc.tile_pool(name="const", bufs=1) as cp:
        # Persistent small tensors (unique tags -> each gets its own slot).
        def ctile(shape, dt, name):
            return cp.tile(shape, dt, name=name, tag=name)

        b1_sb = ctile([128, 2], f32, "b1_sb")
        w2_sb = ctile([128, 128], bf16, "w2_sb")
        b2_sb = ctile([32, 1], f32, "b2_sb")
        w3_sb = ctile([32, 32], bf16, "w3_sb")
        b3_sb = ctile([32, 1], f32, "b3_sb")
        w4_sb = ctile([32, 1], bf16, "w4_sb")
        b4_sb = ctile([1, 1], f32, "b4_sb")
        a_sb = ctile([128, 2 * ncol], bf16, "a_sb")
        h_sb = ctile([32, half], bf16, "h_sb")
        h2_sb = ctile([32, half], bf16, "h2_sb")
        out_sb = ctile([1, half], f32, "out_sb")

        # Constants go on GpSimd (SWDGE) so they never delay the big
        # streaming DMAs on the Sync HWDGE ring.
        nc.gpsimd.dma_start(out=b1_sb[:], in_=b1_d.ap())
        nc.gpsimd.dma_start(out=w2_sb[:], in_=w2_d.ap())
        nc.gpsimd.dma_start(out=b2_sb[:], in_=b2_d.ap())
        nc.gpsimd.dma_start(out=w3_sb[:], in_=w3_d.ap())
        nc.gpsimd.dma_start(out=b3_sb[:], in_=b3_d.ap())
        nc.gpsimd.dma_start(out=w4_sb[:], in_=w4_d.ap())
        nc.gpsimd.dma_start(out=b4_sb[:], in_=b4_d.ap())

        with (
            tc.tile_pool(name="xpool", bufs=6) as xpool,
            tc.tile_pool(name="wpool", bufs=6) as wpool,
            tc.tile_pool(name="psum", bufs=1, space="PSUM") as pp,
        ):
            pa = [
                pp.tile([128, ncol], f32, name=f"pa{m}", tag=f"pa{m}")
                for m in range(2)
            ]

            # Front layer: stream x and W1 K-contiguously, accumulate in PSUM.
            # fp8 DoubleRow: one MATMUL contracts a pair of K-tiles (K=256)
            # via 3D APs [128, 2, N]; the array holds 2 fp8 weights per cell.
            DR = mybir.MatmulPerfMode.DoubleRow
            npair = nkt // 2
            k0 = 0
            for cnt in chunks:
                assert cnt % 2 == 0
                xt = xpool.tile([128, cmax, ncol], fp8, name="xt", tag="xt")
                wt = wpool.tile([128, cmax, O1], fp8, name="wt", tag="wt")
                x_ap = x_d.ap()[k0 * 128 : (k0 + cnt) * 128].rearrange(
                    "(p j) c -> p j c", p=128
                )
                w_ap = w_d.ap()[k0 * 128 : (k0 + cnt) * 128].rearrange(
                    "(p j) c -> p j c", p=128
                )
                nc.sync.dma_start(out=xt[:, :cnt, :], in_=x_ap)
                nc.scalar.dma_start(out=wt[:, :cnt, :], in_=w_ap)
                for t in range(cnt // 2):
                    kp = k0 // 2 + t
                    rhs = xt[:, 2 * t : 2 * t + 2, :]
                    for m in range(2):
                        lhsT = wt[:, 2 * t : 2 * t + 2, m * 128 : (m + 1) * 128]
                        nc.tensor.matmul(
                            pa[m][:],
                            lhsT,
                            rhs,
                            start=(kp == 0),
                            stop=(kp == npair - 1),
                            perf_mode=DR,
                        )
                k0 += cnt

            # a = relu(front/W1_SCALE + b1), cast to bf16.
            # Partition = output neuron.
            for m in range(2):
                nc.scalar.activation(
                    out=a_sb[:, m * ncol : (m + 1) * ncol],
                    in_=pa[m][:],
                    func=Act.Relu,
                    bias=b1_sb[:, m : m + 1],
                    scale=1.0 / W1_SCALE,
                )

            # Layer 2: K = 512 over concat(a_0, a_1); 4 K-tiles of 128.
            p2 = pp.tile([32, half], f32, name="p2", tag="p2")
            for j in range(4):
                m, ppi = j % 2, j // 2
                rhs = a_sb[:, m * ncol + ppi * half : m * ncol + ppi * half + half]
                nc.tensor.matmul(
                    p2[:],
                    w2_sb[:, j * 32 : (j + 1) * 32],
                    rhs,
                    start=(j == 0),
                    stop=(j == 3),
                )
            nc.scalar.activation(out=h_sb[:], in_=p2[:], func=Act.Relu, bias=b2_sb[:])

            # Layer 3.
            p3 = pp.tile([32, half], f32, name="p3", tag="p3")
            nc.tensor.matmul(p3[:], w3_sb[:], h_sb[:], start=True, stop=True)
            nc.scalar.activation(out=h2_sb[:], in_=p3[:], func=Act.Relu, bias=b3_sb[:])

            # Layer 4 + sigmoid.
            p4 = pp.tile([1, half], f32, name="p4", tag="p4")
            nc.tensor.matmul(p4[:], w4_sb[:], h2_sb[:], start=True, stop=True)
            nc.scalar.activation(
                out=out_sb[:], in_=p4[:], func=Act.Sigmoid, bias=b4_sb[:]
            )

            nc.sync.dma_start(out=out_d.ap(), in_=out_sb[:])

    nc.compile()
    return nc


def _pack_chunks(t_k, chunks):
    """[nkt, 128, C] k-tile-major -> packed [(sum cnt)*128, C] where each
    chunk block is [128, cnt, C] flattened (partition-major)."""
    blocks = []
    k0 = 0
    for cnt in chunks:
        blk = t_k[k0 : k0 + cnt].transpose(1, 0, 2)  # [128, cnt, C]
        blocks.append(np.ascontiguousarray(blk).reshape(128 * cnt, -1))
        k0 += cnt
    return np.concatenate(blocks, axis=0)


def _prep_inputs(x, W1, b1, W2, b2, W3, b3, W4, b4, chunks, ncol):
    """Host-side shard + layout + bf16 cast. Returns per-core input maps."""
    n_cores = N_CORES
    nkt = sum(chunks)
    bc = x.shape[0] // n_cores
    o1 = W1.shape[0]

    # x: [B, 2, F] -> per core k-tile-major [nkt, 128, ncol] with
    # element [k, p, pp*bc + b] = x[core*bc + b, pp, k*128 + p],
    # then packed per chunk.
    xb = np.asarray(x, dtype=FP8)
    xr = xb.reshape(n_cores, bc, PERSP, nkt, 128)
    xk = np.ascontiguousarray(xr.transpose(0, 3, 4, 2, 1)).reshape(
        n_cores, nkt, 128, ncol
    )
    xh = np.stack([_pack_chunks(xk[c], chunks) for c in range(n_cores)])

    # W1: [256, F] -> [nkt, 128, 256], element [k, p, o] = W1[o, k*128 + p].
    # Pre-scaled so the fp8 exponent range is used well.
    wb = (np.asarray(W1, np.float32) * W1_SCALE).astype(FP8)
    wk = np.ascontiguousarray(wb.reshape(o1, nkt, 128).transpose(1, 2, 0))
    wh = _pack_chunks(wk, chunks)

    b1h = np.ascontiguousarray(np.asarray(b1, np.float32).reshape(2, 128).T)

    # W2: [32, 512] -> W2T tiled [128, 4*32]
    w2h = np.ascontiguousarray(
        np.asarray(W2, dtype=BF16).T.reshape(4, 128, 32).transpose(1, 0, 2)
    ).reshape(128, 128)
    b2h = np.asarray(b2, np.float32).reshape(32, 1)
    w3h = np.ascontiguousarray(np.asarray(W3, dtype=BF16).T)
    b3h = np.asarray(b3, np.float32).reshape(32, 1)
    w4h = np.ascontiguousarray(np.asarray(W4, dtype=BF16).T)
    b4h = np.asarray(b4, np.float32).reshape(1, 1)

    in_maps = []
    for c in range(n_cores):
        in_maps.append(
            {
                "x": xh[c],
                "w1": wh,
                "b1": b1h,
                "w2": w2h,
                "b2": b2h,
                "w3": w3h,
                "b3": b3h,
                "w4": w4h,
                "b4": b4h,
            }
        )
    return in_maps


_RUN_KW = {}  # test.py can inject trace kwargs here
_LAST_RESULT = [None]  # test.py reads profiling info back


def kernel(x, W1, b1, W2, b2, W3, b3, W4, b4):
    from concourse import bass_utils

    nc = _build(CHUNKS, NKT, NCOL)
    in_maps = _prep_inputs(x, W1, b1, W2, b2, W3, b3, W4, b4, CHUNKS, NCOL)
    res = bass_utils.run_bass_kernel_spmd(
        nc, in_maps, core_ids=list(range(N_CORES)), **_RUN_KW
    )
    _LAST_RESULT[0] = res
    out = np.empty((B, 1), dtype=np.float32)
    for c in range(N_CORES):
        out[c * BC : (c + 1) * BC, 0] = res.results[c]["out"][0]
    return out


# revision 16
# speedup vs baseline: 1.1563x; 1.1563x over previous
"""NNUE forward kernel for Trainium2 (8 NeuronCores, batch-parallel).

Network (per batch row b, dual perspective p in {0,1}):
    a_p = relu(x[b,p,:] @ W1.T + b1)        # [256], K = 45056
    a   = concat(a_0, a_1)                  # [512]
    h   = relu(a @ W2.T + b2)               # [32]
    h2  = relu(h @ W3.T + b3)               # [32]
    y   = sigmoid(h2 @ W4.T + b4)           # [1]

Sharding: data-parallel over batch.  Each core takes 256 batch rows
(= 512 matmul columns counting both perspectives, which share W1).  W1
and the tiny MLP weights are replicated.  All matmul operands are cast
to bf16 on the host (fp32 PSUM accumulation on device), which halves
HBM traffic and runs the PE at full bf16 rate.

On-device layout: features live on the partition axis (prepared
host-side, partition-major in DRAM so every DMA is one contiguous run
per partition).  The front layer is out[o, col] = sum_f W1T[f, o] *
xT[f, col], 352 K-tiles of 128 accumulating into two PSUM banks
([128 outs, 512 cols] each).  The K stream is chunked with small chunks
first so the PE starts as early as possible.  The MLP tail keeps batch
on the free axis, so no on-device transposes are needed anywhere.
"""

import numpy as np
import ml_dtypes

BF16 = ml_dtypes.bfloat16
FP8 = ml_dtypes.float8_e4m3
W1_SCALE = 256.0  # W1 ~ 1/sqrt(45056) sits near fp8 subnormals; pre-scale up

N_CORES = 8

# Full-problem geometry (hardcoded per the harness contract).
B = 2048          # total batch
PERSP = 2
F = 45056         # input features = 352 * 128
O1 = 256          # front layer width
NKT = F // 128    # 352 K-tiles
BC = B // N_CORES    # 256 batch rows per core
NCOL = BC * PERSP    # 512 matmul columns per core
# K-tiles per DMA chunk: small chunks first to cut PE startup latency,
# then big chunks so each fp8 transfer stays >= 1-2 MB for DMA efficiency.
CHUNKS = [4, 4, 8, 16] + [32] * ((NKT - 32) // 32)
assert sum(CHUNKS) == NKT


def _build(chunks, nkt, ncol, num_devices=N_CORES):
    """Emit the Bass program. Returns the compiled Bacc object."""
    import concourse.mybir as mybir
    from concourse import bacc
    from concourse.tile import TileContext

    f32 = mybir.dt.float32
    bf16 = mybir.dt.bfloat16
    fp8 = mybir.dt.float8e4
    Act = mybir.ActivationFunctionType

    half = ncol // 2   # columns per perspective
    cmax = max(chunks)

    nc = bacc.Bacc(
        "TRN2", target_bir_lowering=False, debug=False, num_devices=num_devices
    )

    # Chunk blocks are packed sequentially in DRAM (each block is
    # [128, cnt, ncol] flattened row-major) so every streaming DMA reads
    # one dense region of HBM.
    x_d = nc.dram_tensor("x", [nkt * 128, ncol], fp8, kind="ExternalInput")
    w_d = nc.dram_tensor("w1", [nkt * 128, O1], fp8, kind="ExternalInput")
    b1_d = nc.dram_tensor("b1", [128, 2], f32, kind="ExternalInput")
    w2_d = nc.dram_tensor("w2", [128, 128], bf16, kind="ExternalInput")
    b2_d = nc.dram_tensor("b2", [32, 1], f32, kind="ExternalInput")
    w3_d = nc.dram_tensor("w3", [32, 32], bf16, kind="ExternalInput")
    b3_d = nc.dram_tensor("b3", [32, 1], f32, kind="ExternalInput")
    w4_d = nc.dram_tensor("w4", [32, 1], bf16, kind="ExternalInput")
    b4_d = nc.dram_tensor("b4", [1, 1], f32, kind="ExternalInput")
    out_d = nc.dram_tensor("out", [1, half], f32, kind="ExternalOutput")

    with TileContext(nc) as tc, tc.tile_pool(name="const", bufs=1) as cp:
        # Persistent small tensors (unique tags -> each gets its own slot).
        def ctile(shape, dt, name):
            return cp.tile(shape, dt, name=name, tag=name)

        b1_sb = ctile([128, 2], f32, "b1_sb")
        w2_sb = ctile([128, 128], bf16, "w2_sb")
        b2_sb = ctile([32, 1], f32, "b2_sb")
        w3_sb = ctile([32, 32], bf16, "w3_sb")
        b3_sb = ctile([32, 1], f32, "b3_sb")
        w4_sb = ctile([32, 1], bf16, "w4_sb")
        b4_sb = ctile([1, 1], f32, "b4_sb")
        a_sb = ctile([128, 2 * ncol], bf16, "a_sb")
        h_sb = ctile([32, half], bf16, "h_sb")
        h2_sb = ctile([32, half], bf16, "h2_sb")
        out_sb = ctile([1, half], f32, "out_sb")

        # Constants go on GpSimd (SWDGE) so they never delay the big
        # streaming DMAs on the Sync HWDGE ring.
        nc.gpsimd.dma_start(out=b1_sb[:], in_=b1_d.ap())
        nc.gpsimd.dma_start(out=w2_sb[:], in_=w2_d.ap())
        nc.gpsimd.dma_start(out=b2_sb[:], in_=b2_d.ap())
        nc.gpsimd.dma_start(out=w3_sb[:], in_=w3_d.ap())
        nc.gpsimd.dma_start(out=b3_sb[:], in_=b3_d.ap())
        nc.gpsimd.dma_start(out=w4_sb[:], in_=w4_d.ap())
        nc.gpsimd.dma_start(out=b4_sb[:], in_=b4_d.ap())

        with (
            tc.tile_pool(name="xpool", bufs=4) as xpool,
            tc.tile_pool(name="wpool", bufs=4) as wpool,
            tc.tile_pool(name="psum", bufs=1, space="PSUM") as pp,
        ):
            pa = [
                pp.tile([128, ncol], f32, name=f"pa{m}", tag=f"pa{m}")
                for m in range(2)
            ]

            # Front layer: stream x and W1 K-contiguously, accumulate in PSUM.
            # fp8 DoubleRow: one MATMUL contracts a pair of K-tiles (K=256)
            # via 3D APs [128, 2, N]; the array holds 2 fp8 weights per cell.
            DR = mybir.MatmulPerfMode.DoubleRow
            npair = nkt // 2
            k0 = 0
            for cnt in chunks:
                assert cnt % 2 == 0
                xt = xpool.tile([128, cmax, ncol], fp8, name="xt", tag="xt")
                wt = wpool.tile([128, cmax, O1], fp8, name="wt", tag="wt")
                x_ap = x_d.ap()[k0 * 128 : (k0 + cnt) * 128].rearrange(
                    "(p j) c -> p j c", p=128
                )
                w_ap = w_d.ap()[k0 * 128 : (k0 + cnt) * 128].rearrange(
                    "(p j) c -> p j c", p=128
                )
                nc.sync.dma_start(out=xt[:, :cnt, :], in_=x_ap)
                nc.scalar.dma_start(out=wt[:, :cnt, :], in_=w_ap)
                for t in range(cnt // 2):
                    kp = k0 // 2 + t
                    rhs = xt[:, 2 * t : 2 * t + 2, :]
                    for m in range(2):
                        lhsT = wt[:, 2 * t : 2 * t + 2, m * 128 : (m + 1) * 128]
                        nc.tensor.matmul(
                            pa[m][:],
                            lhsT,
                            rhs,
                            start=(kp == 0),
                            stop=(kp == npair - 1),
                            perf_mode=DR,
                        )
                k0 += cnt

            # a = relu(front/W1_SCALE + b1), cast to bf16.
            # Partition = output neuron.
            for m in range(2):
                nc.scalar.activation(
                    out=a_sb[:, m * ncol : (m + 1) * ncol],
                    in_=pa[m][:],
                    func=Act.Relu,
                    bias=b1_sb[:, m : m + 1],
                    scale=1.0 / W1_SCALE,
                )

            # Layer 2: K = 512 over concat(a_0, a_1); 4 K-tiles of 128.
            p2 = pp.tile([32, half], f32, name="p2", tag="p2")
            for j in range(4):
                m, ppi = j % 2, j // 2
                rhs = a_sb[:, m * ncol + ppi * half : m * ncol + ppi * half + half]
                nc.tensor.matmul(
                    p2[:],
                    w2_sb[:, j * 32 : (j + 1) * 32],
                    rhs,
                    start=(j == 0),
                    stop=(j == 3),
                )
            nc.scalar.activation(out=h_sb[:], in_=p2[:], func=Act.Relu, bias=b2_sb[:])

            # Layer 3.
            p3 = pp.tile([32, half], f32, name="p3", tag="p3")
            nc.tensor.matmul(p3[:], w3_sb[:], h_sb[:], start=True, stop=True)
            nc.scalar.activation(out=h2_sb[:], in_=p3[:], func=Act.Relu, bias=b3_sb[:])

            # Layer 4 + sigmoid.
            p4 = pp.tile([1, half], f32, name="p4", tag="p4")
            nc.tensor.matmul(p4[:], w4_sb[:], h2_sb[:], start=True, stop=True)
            nc.scalar.activation(
                out=out_sb[:], in_=p4[:], func=Act.Sigmoid, bias=b4_sb[:]
            )

            nc.sync.dma_start(out=out_d.ap(), in_=out_sb[:])

    nc.compile()
    return nc


def _pack_chunks(t_k, chunks):
    """[nkt, 128, C] k-tile-major -> packed [(sum cnt)*128, C] where each
    chunk block is [128, cnt, C] flattened (partition-major)."""
    blocks = []
    k0 = 0
    for cnt in chunks:
        blk = t_k[k0 : k0 + cnt].transpose(1, 0, 2)  # [128, cnt, C]
        blocks.append(np.ascontiguousarray(blk).reshape(128 * cnt, -1))
        k0 += cnt
    return np.concatenate(blocks, axis=0)


def _prep_inputs(x, W1, b1, W2, b2, W3, b3, W4, b4, chunks, ncol):
    """Host-side shard + layout + bf16 cast. Returns per-core input maps."""
    n_cores = N_CORES
    nkt = sum(chunks)
    bc = x.shape[0] // n_cores
    o1 = W1.shape[0]

    # x: [B, 2, F] -> per core k-tile-major [nkt, 128, ncol] with
    # element [k, p, pp*bc + b] = x[core*bc + b, pp, k*128 + p],
    # then packed per chunk.
    xb = np.asarray(x, dtype=FP8)
    xr = xb.reshape(n_cores, bc, PERSP, nkt, 128)
    xk = np.ascontiguousarray(xr.transpose(0, 3, 4, 2, 1)).reshape(
        n_cores, nkt, 128, ncol
    )
    xh = np.stack([_pack_chunks(xk[c], chunks) for c in range(n_cores)])

    # W1: [256, F] -> [nkt, 128, 256], element [k, p, o] = W1[o, k*128 + p].
    # Pre-scaled so the fp8 exponent range is used well.
    wb = (np.asarray(W1, np.float32) * W1_SCALE).astype(FP8)
    wk = np.ascontiguousarray(wb.reshape(o1, nkt, 128).transpose(1, 2, 0))
    wh = _pack_chunks(wk, chunks)

    b1h = np.ascontiguousarray(np.asarray(b1, np.float32).reshape(2, 128).T)

    # W2: [32, 512] -> W2T tiled [128, 4*32]
    w2h = np.ascontiguousarray(
        np.asarray(W2, dtype=BF16).T.reshape(4, 128, 32).transpose(1, 0, 2)
    ).reshape(128, 128)
    b2h = np.asarray(b2, np.float32).reshape(32, 1)
    w3h = np.ascontiguousarray(np.asarray(W3, dtype=BF16).T)
    b3h = np.asarray(b3, np.float32).reshape(32, 1)
    w4h = np.ascontiguousarray(np.asarray(W4, dtype=BF16).T)
    b4h = np.asarray(b4, np.float32).reshape(1, 1)

    in_maps = []
    for c in range(n_cores):
        in_maps.append(
            {
                "x": xh[c],
                "w1": wh,
                "b1": b1h,
                "w2": w2h,
                "b2": b2h,
                "w3": w3h,
                "b3": b3h,
                "w4": w4h,
                "b4": b4h,
            }
        )
    return in_maps


_RUN_KW = {}  # test.py can inject trace kwargs here
_LAST_RESULT = [None]  # test.py reads profiling info back


def kernel(x, W1, b1, W2, b2, W3, b3, W4, b4):
    from concourse import bass_utils

    nc = _build(CHUNKS, NKT, NCOL)
    in_maps = _prep_inputs(x, W1, b1, W2, b2, W3, b3, W4, b4, CHUNKS, NCOL)
    res = bass_utils.run_bass_kernel_spmd(
        nc, in_maps, core_ids=list(range(N_CORES)), **_RUN_KW
    )
    _LAST_RESULT[0] = res
    out = np.empty((B, 1), dtype=np.float32)
    for c in range(N_CORES):
        out[c * BC : (c + 1) * BC, 0] = res.results[c]["out"][0]
    return out
